# revision 1
# baseline (speedup 1.0000x reference)
"""Trainium2 Bass kernel for nn_LocalConnectivity (diamond-ring circular stencil).

out[i,j] = sum_{d=1..5} w_d * sum_{|di|+|dj|=d} x[(i+di)%H, (j+dj)%W]

Strategy: row-shard across 8 NeuronCores (512 rows each + 5-row circular
halo, columns pre-padded with 5-col circular halo on host). Per core the
61-tap stencil is computed on the TensorEngine as 11 banded matmuls (one
per column shift dj in [-5,5]): PSUM[m, c] += W_dj[k, m] * strip[k, c+5+dj]
where W_dj is a [128, 118] constant band matrix holding the vertical taps
for that dj and the column shift rides the rhs access pattern for free.
float32r matmuls stream at 1 cycle/row (vs 4 for float32) at ~2e-4 rel err.
"""
import numpy as np
from contextlib import ExitStack

import concourse.bass as bass
import concourse.tile as tile
from concourse import bacc, mybir
from concourse.bass_utils import run_bass_kernel_spmd

N_CORES = 8
H = W = 4096
MAXD = 5
ROWS_PER_CORE = H // N_CORES          # 512
IN_ROWS = ROWS_PER_CORE + 2 * MAXD    # 522
IN_COLS = W + 2 * MAXD                # 4106
NCOL = 512                            # matmul free dim (one PSUM bank, fp32 max)
NCHUNK = W // NCOL                    # 8
M_OUT = 118                           # output rows per row-window (K=128 - 2*MAXD)
# row windows: (input_row_start, out_row_start, K, M)
WINDOWS = []
_o = 0
while _o < ROWS_PER_CORE:
    m = min(M_OUT, ROWS_PER_CORE - _o)
    WINDOWS.append((_o, _o, m + 2 * MAXD, m))
    _o += m

_CACHE = {}


def _band_weights(distance_weights: np.ndarray) -> np.ndarray:
    """w_flat [128, 11*118]: w_flat[k, (dj+5)*118 + m] = K2d[k-m-5, dj]."""
    wd = np.asarray(distance_weights, dtype=np.float32)
    w = np.zeros((11, 128, M_OUT), dtype=np.float32)
    for dj in range(-MAXD, MAXD + 1):
        for di in range(-MAXD, MAXD + 1):
            d = abs(di) + abs(dj)
            if not (1 <= d <= MAXD):
                continue
            m = np.arange(M_OUT)
            k = m + MAXD + di
            ok = (k >= 0) & (k < 128)
            w[dj + MAXD, k[ok], m[ok]] = wd[d - 1]
    return np.ascontiguousarray(w.transpose(1, 0, 2).reshape(128, 11 * M_OUT))


def _build():
    dtr = mybir.dt.float32r
    dtf = mybir.dt.float32
    nc = bacc.Bacc("TRN2", target_bir_lowering=False, debug=False,
                   num_devices=N_CORES)
    x = nc.dram_tensor("x", [IN_ROWS, IN_COLS], dtr, kind="ExternalInput").ap()
    wts = nc.dram_tensor("w", [128, 11 * M_OUT], dtr, kind="ExternalInput").ap()
    y = nc.dram_tensor("y", [ROWS_PER_CORE, W], dtf, kind="ExternalOutput").ap()

    with tile.TileContext(nc) as tc, ExitStack() as ctx:
        spool = ctx.enter_context(tc.tile_pool(name="strip", bufs=3))
        wpool = ctx.enter_context(tc.tile_pool(name="wts", bufs=1))
        opool = ctx.enter_context(tc.tile_pool(name="out", bufs=2))
        ppool = ctx.enter_context(tc.tile_pool(name="ps", bufs=8, space="PSUM"))

        CMID = IN_COLS // 2
        strips = []
        # Issue strip0 before the weights so the critical first window's
        # data transfer starts immediately; weights ride the idle sync queue.
        for wi, (in0, out0, kdim, m) in enumerate(WINDOWS):
            if wi == 0:
                st = spool.tile([128, IN_COLS], dtr, tag="strip")
                nc.gpsimd.dma_start(st[:kdim, :CMID], x[in0:in0 + kdim, :CMID])
                nc.scalar.dma_start(st[:kdim, CMID:], x[in0:in0 + kdim, CMID:])
                strips.append(st)
        wt = wpool.tile([128, 11 * M_OUT], dtr)
        nc.sync.dma_start(wt[:], wts[:])

        for wi, (in0, out0, kdim, m) in enumerate(WINDOWS):
            if wi == 0:
                st = strips[0]
            else:
                st = spool.tile([128, IN_COLS], dtr, tag="strip")
                nc.gpsimd.dma_start(st[:kdim, :CMID], x[in0:in0 + kdim, :CMID])
                nc.scalar.dma_start(st[:kdim, CMID:], x[in0:in0 + kdim, CMID:])
            ot = opool.tile([m, W], dtf, tag="out")
            for cc in range(NCHUNK):
                ps = ppool.tile([m, NCOL], dtf, tag="ps")
                for j, dj in enumerate(range(-MAXD, MAXD + 1)):
                    c0 = cc * NCOL + MAXD + dj
                    nc.tensor.matmul(
                        ps[:],
                        wt[:kdim, (dj + MAXD) * M_OUT:(dj + MAXD) * M_OUT + m],
                        st[:kdim, c0:c0 + NCOL],
                        start=(j == 0), stop=(j == 10),
                    )
                dst = ot[:, cc * NCOL:(cc + 1) * NCOL]
                if cc % 2 == 0:
                    nc.vector.tensor_copy(dst, ps[:])
                else:
                    nc.scalar.copy(dst, ps[:])
            # One fully-contiguous DRAM write per window (m full rows) so the
            # HW DGE fans it out across all 16 SDMA engines; keep stores off
            # the strip queues to avoid head-of-line blocking the prefetch.
            nc.sync.dma_start(y[out0:out0 + m, :], ot[:])
    nc.compile()
    return nc


def kernel(grid_spikes: np.ndarray, distance_weights: np.ndarray) -> np.ndarray:
    x = np.ascontiguousarray(grid_spikes, dtype=np.float32)
    assert x.shape == (H, W)
    if "nc" not in _CACHE:
        _CACHE["nc"] = _build()
    nc = _CACHE["nc"]

    w_flat = _band_weights(distance_weights)
    xpad = np.concatenate([x[:, -MAXD:], x, x[:, :MAXD]], axis=1)
    in_maps = []
    for c in range(N_CORES):
        rows = np.arange(c * ROWS_PER_CORE - MAXD,
                         c * ROWS_PER_CORE + ROWS_PER_CORE + MAXD) % H
        in_maps.append({"x": np.ascontiguousarray(xpad[rows]), "w": w_flat})

    res = run_bass_kernel_spmd(nc, in_maps, list(range(N_CORES)))
    out = np.concatenate([res.results[c]["y"] for c in range(N_CORES)], axis=0)
    return out.astype(np.float32)



# revision 5
# speedup vs baseline: 1.6634x; 1.6634x over previous
"""Trainium2 Bass kernel for nn_LocalConnectivity (diamond-ring circular stencil).

out[i,j] = sum_{d=1..5} w_d * sum_{|di|+|dj|=d} x[(i+di)%H, (j+dj)%W]

Strategy: row-shard across 8 NeuronCores (512 rows each + 5-row circular
halo, columns pre-padded with 5-col circular halo on host). Per core the
60-tap stencil runs on the TensorEngine as 11 banded matmuls (one per
column shift dj in [-5,5]): PSUM[m, c] += W_dj[k, m] * strip[k, c+5+dj].

All data is bf16 (error ~4e-3 << 2e-2 gate), halving HBM traffic vs fp32.
Loop order is dj-outer / chunk-inner so the stationary band is reused
across the 8 PSUM banks (55 LDWEIGHTS instead of 440). Input strips keep
8212-byte DMA lines; the output is split across 4 engine queues so the
HW DGE fans writes over many SDMA engines (the old kernel's writes
landed on 2 engines and dominated the critical path).
"""
import numpy as np
from contextlib import ExitStack

import ml_dtypes

import concourse.bass as bass
import concourse.tile as tile
from concourse import bacc, mybir
from concourse.bass_utils import run_bass_kernel_spmd

N_CORES = 8
H = W = 4096
MAXD = 5
ROWS_PER_CORE = H // N_CORES          # 512
IN_ROWS = ROWS_PER_CORE + 2 * MAXD    # 522
IN_COLS = W + 2 * MAXD                # 4106
NCOL = 512                            # matmul free dim (one PSUM bank, fp32 max)
NCHUNK = W // NCOL                    # 8
M_OUT = 118                           # output rows per row-window (K=128 - 2*MAXD)
# row windows: (out_row_start, K, M)
WINDOWS = []
_o = 0
while _o < ROWS_PER_CORE:
    _m = min(M_OUT, ROWS_PER_CORE - _o)
    WINDOWS.append((_o, _m + 2 * MAXD, _m))
    _o += _m

_CACHE = {}


def _band_weights(distance_weights: np.ndarray) -> np.ndarray:
    """w_flat [128, 11*118] bf16: w_flat[k, (dj+5)*118 + m] = K2d[k-m-5, dj]."""
    wd = np.asarray(distance_weights, dtype=np.float32)
    w = np.zeros((11, 128, M_OUT), dtype=np.float32)
    for dj in range(-MAXD, MAXD + 1):
        for di in range(-MAXD, MAXD + 1):
            d = abs(di) + abs(dj)
            if not (1 <= d <= MAXD):
                continue
            m = np.arange(M_OUT)
            k = m + MAXD + di
            ok = (k >= 0) & (k < 128)
            w[dj + MAXD, k[ok], m[ok]] = wd[d - 1]
    out = w.transpose(1, 0, 2).reshape(128, 11 * M_OUT)
    return np.ascontiguousarray(out.astype(ml_dtypes.bfloat16))


def _build():
    dtb = mybir.dt.bfloat16
    dtf = mybir.dt.float32
    nc = bacc.Bacc("TRN2", target_bir_lowering=False, debug=False,
                   num_devices=N_CORES)
    x = nc.dram_tensor("x", [IN_ROWS, IN_COLS], dtb, kind="ExternalInput").ap()
    wts = nc.dram_tensor("w", [128, 11 * M_OUT], dtb, kind="ExternalInput").ap()
    y = nc.dram_tensor("y", [ROWS_PER_CORE, W], dtb, kind="ExternalOutput").ap()

    with tile.TileContext(nc) as tc, ExitStack() as ctx:
        spool = ctx.enter_context(tc.tile_pool(name="strip", bufs=len(WINDOWS)))
        wpool = ctx.enter_context(tc.tile_pool(name="wts", bufs=1))
        opool = ctx.enter_context(tc.tile_pool(name="out", bufs=3))
        ppool = ctx.enter_context(tc.tile_pool(name="ps", bufs=8, space="PSUM"))

        # Prefetch every strip up front (all fit in SBUF in bf16); rows are
        # split between two queues to double the descriptor fan-out while
        # keeping full 8212-byte DMA lines.
        strips = []
        wt = None
        for wi, (out0, kdim, m) in enumerate(WINDOWS):
            st = spool.tile([128, IN_COLS], dtb, tag="strip")
            h = kdim // 2
            nc.sync.dma_start(st[:h, :], x[out0:out0 + h, :])
            nc.gpsimd.dma_start(st[h:kdim, :], x[out0 + h:out0 + kdim, :])
            strips.append(st)
            if wi == 0:
                wt = wpool.tile([128, 11 * M_OUT], dtb)
                nc.scalar.dma_start(wt[:], wts[:])

        oqueues = None
        for wi, (out0, kdim, m) in enumerate(WINDOWS):
            st = strips[wi]
            ot = opool.tile([m, W], dtb, tag="out")
            pss = [ppool.tile([m, NCOL], dtf, tag="ps", name=f"ps{wi}_{i}")
                   for i in range(NCHUNK)]
            # dj-outer so the stationary band is loaded once per dj and
            # reused across all 8 chunks (PSUM banks).
            for j, dj in enumerate(range(-MAXD, MAXD + 1)):
                wsl = wt[:kdim, (dj + MAXD) * M_OUT:(dj + MAXD) * M_OUT + m]
                for cc in range(NCHUNK):
                    c0 = cc * NCOL + MAXD + dj
                    nc.tensor.matmul(
                        pss[cc][:], wsl, st[:kdim, c0:c0 + NCOL],
                        start=(j == 0), stop=(j == 10),
                    )
            for cc in range(NCHUNK):
                dst = ot[:, cc * NCOL:(cc + 1) * NCOL]
                if cc % 2 == 0:
                    nc.vector.tensor_copy(dst, pss[cc][:])
                else:
                    nc.scalar.copy(dst, pss[cc][:])
            # Output rows split across the 3 DMA-capable engine queues so
            # the HW DGE fans writes over more SDMA engines.
            qs = [nc.sync, nc.gpsimd, nc.scalar]
            splits = np.linspace(0, m, len(qs) + 1).astype(int)
            for qi, q in enumerate(qs):
                r0, r1 = int(splits[qi]), int(splits[qi + 1])
                if r1 > r0:
                    q.dma_start(y[out0 + r0:out0 + r1, :], ot[r0:r1, :])
    nc.compile()
    return nc


def _prep(grid_spikes: np.ndarray, distance_weights: np.ndarray):
    """Build the per-core input maps (bf16 slabs + band weights)."""
    x = np.ascontiguousarray(grid_spikes, dtype=np.float32)
    assert x.shape == (H, W)
    w_flat = _band_weights(distance_weights)
    xpad = np.concatenate([x[:, -MAXD:], x, x[:, :MAXD]],
                          axis=1).astype(ml_dtypes.bfloat16)
    in_maps = []
    for c in range(N_CORES):
        rows = np.arange(c * ROWS_PER_CORE - MAXD,
                         c * ROWS_PER_CORE + ROWS_PER_CORE + MAXD) % H
        in_maps.append({"x": np.ascontiguousarray(xpad[rows]), "w": w_flat})
    return in_maps


def kernel(grid_spikes: np.ndarray, distance_weights: np.ndarray) -> np.ndarray:
    if "nc" not in _CACHE:
        _CACHE["nc"] = _build()
    nc = _CACHE["nc"]
    in_maps = _prep(grid_spikes, distance_weights)
    res = run_bass_kernel_spmd(nc, in_maps, list(range(N_CORES)))
    out = np.concatenate([res.results[c]["y"] for c in range(N_CORES)], axis=0)
    return out.astype(np.float32)
